# revision 3
# baseline (speedup 1.0000x reference)
"""Trainium2 Bass kernel for the CRF problem.

Math:
  feat = conv2d(X.view(-1,1,16,8), K, pad=2)  -> flatten      (B, L, D)
  e    = feat @ W                                              (B, L, Y)
Both are linear in X, so fold:  e = X @ G  with  G = C_K @ W  (D x Y),
C_K the 128x128 conv matrix built from the 5x5 kernel (host prep, tiny).

logZ via the *scaled* forward algorithm:
  A_0 = exp(e_0);  A_t = exp(e_t) * (A_{t-1} @ M),  M = exp(T)/Y
  logZ = log(sum_y A_{L-1}) + (L-1)*log(Y)
(per-step matvec with the constant 26x26 matrix M == one PE matmul).

Per-core layout (512 words/core = 4 groups x 128 words):
  partitions = 32*g + y (y<26, rows 26..31 zero-padded), free = words.
  e^T produced by matmul(lhsT=G64 fp8, rhs=X^T fp8 chunk) into psum
  (scaled by 64 for fp8 fidelity; the Exp activation rescales by 1/64).
  Forward chain multiplies ride the DVE, backward chain rides GpSimd,
  so neither engine saturates during the serial recursion.

em-score and the label-pair transition score are linear functionals of
(X, labels) with tiny outputs; the host computes them exactly (fp64
accumulate) while the device handles the only part that needs the
recursion: logZ.  Device output per core: LNS (4,128) = log(sum_y A)
per (group, word).
"""

import numpy as np
import ml_dtypes

B, L, D, Y = 4096, 64, 128, 26
NCORES = 8
WPC = B // NCORES          # 512 words per core
NG, GW = 4, 128            # word groups per core
NTAU, TT = 8, 8            # t-blocks x t-per-block (NTAU*TT == L)
C_REG = 1000.0
GSCALE = 64.0              # fp8 pre-scale on G, undone in the Exp

_BF16 = ml_dtypes.bfloat16
_FP8 = ml_dtypes.float8_e4m3
_PROG = {}


def _conv_matrix(K5):
    """C[q, p]: flattened-input q contribution to flattened-output p."""
    H, Wd = 16, 8
    C = np.zeros((D, D), dtype=np.float64)
    for oh in range(H):
        for ow in range(Wd):
            p = oh * Wd + ow
            for kh in range(5):
                for kw in range(5):
                    ih, iw = oh + kh - 2, ow + kw - 2
                    if 0 <= ih < H and 0 <= iw < Wd:
                        C[ih * Wd + iw, p] = K5[kh, kw]
    return C


def _build_program(reps=1):
    if reps in _PROG:
        return _PROG[reps]
    import concourse.tile as tile
    import concourse.mybir as mybir
    from concourse import bacc
    from concourse.bass import ds, ts

    f32 = mybir.dt.float32
    bf16 = mybir.dt.bfloat16
    fp8 = mybir.dt.float8e4

    nc = bacc.Bacc("TRN2", target_bir_lowering=False, debug=False,
                   num_devices=NCORES)

    XT_d = nc.dram_tensor("XT", [D, WPC * L], fp8, kind="ExternalInput")
    G64_d = nc.dram_tensor("G64", [D, 32], fp8, kind="ExternalInput")
    BDM_d = nc.dram_tensor("BDM", [128, 128], bf16, kind="ExternalInput")
    BDMT_d = nc.dram_tensor("BDMT", [128, 128], bf16, kind="ExternalInput")
    ONES_d = nc.dram_tensor("ONES4", [128, 4], bf16, kind="ExternalInput")
    LNS_d = nc.dram_tensor("LNS", [4, GW], f32, kind="ExternalOutput")

    TMID = L // 2                 # fwd chain owns t < TMID, bwd owns t >= TMID
    TAU_ORDER = [0, 7, 1, 6, 2, 5, 3, 4]
    CHUNK = NG * GW * TT          # 4096 XT cols per tau
    with tile.TileContext(nc) as tc:
        with (
            tc.tile_pool(name="const", bufs=1) as cpool,
            tc.tile_pool(name="xt", bufs=3) as xtp,
            tc.tile_pool(name="e", bufs=NTAU) as ep,
            tc.tile_pool(name="a", bufs=4) as apool,
            tc.tile_pool(name="out", bufs=1) as opool,
            tc.tile_pool(name="pe", bufs=3, space="PSUM") as pep,
            tc.tile_pool(name="prf", bufs=2, space="PSUM") as prfp,
            tc.tile_pool(name="prb", bufs=2, space="PSUM") as prbp,
            tc.tile_pool(name="pl", bufs=1, space="PSUM") as plp,
        ):
            consts = {}

            def load_consts():
                # via the gpsimd (SWDGE) queue: keeps the SP HWDGE ring free
                # for the data stream
                g64 = cpool.tile([D, 32], fp8)
                nc.gpsimd.dma_start(g64[:], G64_d[:])
                bdm = cpool.tile([128, 128], bf16)
                nc.gpsimd.dma_start(bdm[:], BDM_d[:])
                bdmt = cpool.tile([128, 128], bf16)
                nc.gpsimd.dma_start(bdmt[:], BDMT_d[:])
                ones4 = cpool.tile([128, 4], bf16)
                nc.gpsimd.dma_start(ones4[:], ONES_d[:])
                consts.update(g64=g64, bdm=bdm, bdmt=bdmt, ones4=ones4)

            lns = opool.tile([4, GW], f32)

            for _rep in range(reps):
                e_tiles = {}
                staged = {}

                def produce_dma(tau):
                    xt = xtp.tile([D, CHUNK], fp8)
                    for half in range(2):
                        nc.sync.dma_start(
                            xt[:, ds(half * (CHUNK // 2), CHUNK // 2)],
                            XT_d[:, ds(tau * CHUNK + half * (CHUNK // 2),
                                       CHUNK // 2)])
                    staged[tau] = xt

                banks_done = set()

                def produce_bank(tau, tb):
                    xt = staged[tau]
                    if tau not in e_tiles:
                        e_tiles[tau] = ep.tile([128, GW * TT], bf16,
                                               name="e_t", tag="e_t")
                    e_t = e_tiles[tau]
                    pe = pep.tile([128, 512], f32)
                    for g in range(NG):
                        nc.tensor.matmul(
                            pe[32 * g:32 * g + 32, :],
                            consts["g64"][:],
                            xt[:, ds((tb * NG + g) * 512, 512)],
                            start=True, stop=True,
                            tile_position=(0, 32 * g),
                        )
                    nc.scalar.activation(
                        e_t[:, ds(tb * 512, 512)], pe[:],
                        mybir.ActivationFunctionType.Exp,
                        scale=1.0 / GSCALE,
                    )
                    banks_done.add(2 * tau + tb)
                    if 2 * tau in banks_done and 2 * tau + 1 in banks_done:
                        staged.pop(tau)

                def e_avail(t):
                    return (t // (TT // 2)) in banks_done

                def eslice(t):
                    return e_tiles[t // TT][:, ds((t % TT) * GW, GW)]

                # chain states
                st = {"f": None, "b": None, "ft": 0, "bt": L - 1}

                def fwd_link():
                    # alpha_t = E_t * (alpha_{t-1} @ M)
                    t = st["ft"] + 1
                    pr = prfp.tile([128, GW], f32)
                    nc.tensor.matmul(pr[:], consts["bdm"][:], st["f"],
                                     start=True, stop=True)
                    a_new = apool.tile([128, GW], bf16, tag="af")
                    nc.vector.tensor_mul(a_new[:], pr[:], eslice(t))
                    st["f"] = a_new[:]
                    st["ft"] = t

                def bwd_link():
                    # beta_t = M^T-apply(gamma_{t+1}); gamma_t = E_t * beta_t
                    t = st["bt"] - 1
                    pr = prbp.tile([128, GW], f32)
                    nc.tensor.matmul(pr[:], consts["bdmt"][:], st["b"],
                                     start=True, stop=True)
                    if t == TMID - 1:
                        st["b"] = pr[:]          # beta_31 stays in psum
                    else:
                        g_new = apool.tile([128, GW], bf16, tag="ab")
                        nc.vector.tensor_mul(g_new[:], pr[:], eslice(t))
                        st["b"] = g_new[:]
                    st["bt"] = t

                def drain_chains():
                    # run every link whose E data exists, alternating
                    while True:
                        f_ok = (st["f"] is not None
                                and st["ft"] + 1 < TMID
                                and e_avail(st["ft"] + 1))
                        tb_ = st["bt"] - 1
                        b_ok = (st["b"] is not None
                                and tb_ >= TMID - 1
                                and (tb_ == TMID - 1 or e_avail(tb_)))
                        if not (f_ok or b_ok):
                            return
                        if f_ok:
                            fwd_link()
                        if b_ok:
                            bwd_link()

                produce_dma(TAU_ORDER[0])
                if _rep == 0:
                    load_consts()
                produce_dma(TAU_ORDER[1])
                for k, tau in enumerate(TAU_ORDER):
                    if k + 2 < NTAU:
                        produce_dma(TAU_ORDER[k + 2])
                    for tb in ((0, 1) if tau < NTAU // 2 else (1, 0)):
                        produce_bank(tau, tb)
                        if tau == 0 and tb == 0:
                            st["f"] = e_tiles[0][:, 0:GW]   # alpha_0 = E_0
                        if tau == NTAU - 1 and tb == 1:
                            st["b"] = e_tiles[NTAU - 1][:, ds((TT - 1) * GW,
                                                              GW)]
                        drain_chains()

                # logZ[w] = log( sum_y alpha_31 * beta_31 ) + 63*log(26)
                u = apool.tile([128, GW], bf16, tag="u")
                nc.vector.tensor_mul(u[:], st["b"], st["f"])
                pl = plp.tile([4, GW], f32)
                nc.tensor.matmul(pl[:], consts["ones4"][:], u[:],
                                 start=True, stop=True)
                nc.scalar.activation(lns[:], pl[:],
                                     mybir.ActivationFunctionType.Ln)

            nc.sync.dma_start(LNS_d[:], lns[:])

    nc.compile()
    _PROG[reps] = nc
    return nc


def host_prep(X, labels, W, T, K):
    """Build per-core device inputs + host-side scalars."""
    X = np.asarray(X, dtype=np.float32)
    labels = np.asarray(labels).astype(np.int64)
    W = np.asarray(W, dtype=np.float32)
    T = np.asarray(T, dtype=np.float32)
    K5 = np.asarray(K, dtype=np.float64).reshape(5, 5)

    C = _conv_matrix(K5)
    G = C @ W.astype(np.float64)                    # (D, Y)
    G64b = np.zeros((D, 32), dtype=_FP8)
    G64b[:, :Y] = (G * GSCALE).astype(np.float32).astype(_FP8)

    M = (np.exp(T.astype(np.float64)) / Y).astype(np.float32)
    BDM = np.zeros((128, 128), dtype=_BF16)
    BDMT = np.zeros((128, 128), dtype=_BF16)
    for g in range(NG):
        BDM[32 * g:32 * g + Y, 32 * g:32 * g + Y] = M.astype(_BF16)
        BDMT[32 * g:32 * g + Y, 32 * g:32 * g + Y] = M.T.astype(_BF16)
    ONES = np.zeros((128, 4), dtype=_BF16)
    for g in range(NG):
        ONES[32 * g:32 * g + Y, g] = 1.0

    X8 = X.astype(_FP8)                             # (B, L, D)
    in_maps = []
    for c in range(NCORES):
        Xc = X8[c * WPC:(c + 1) * WPC]              # (512, 64, 128)
        # XT cols: tau-major | (tb,g) chunk | t'*128 + w'
        # Xc view (g, w', tau, tb, t', d) -> (d, tau, tb, g, t', w')
        Xv = Xc.reshape(NG, GW, NTAU, 2, TT // 2, D)
        XT = np.ascontiguousarray(
            Xv.transpose(5, 2, 3, 0, 4, 1)).reshape(D, WPC * L)
        in_maps.append({
            "XT": XT,
            "G64": G64b,
            "BDM": BDM,
            "BDMT": BDMT,
            "ONES4": ONES,
        })

    # host-side exact scalars: emission score (label-bucketed sum of X
    # contracted with G) + transition score
    lab = labels.ravel()
    X2 = X.reshape(-1, D)
    cnt = np.bincount(lab, minlength=Y)
    if cnt.min() > 0:
        perm = np.argsort(lab, kind="stable")
        starts = np.zeros(Y, dtype=np.int64)
        starts[1:] = np.cumsum(cnt)[:-1]
        S = np.add.reduceat(X2[perm], starts, axis=0)   # (Y, D)
    else:
        S = np.zeros((Y, D), dtype=np.float64)
        np.add.at(S, lab, X2)
    em_total = float((S.astype(np.float64) * G.T).sum())
    tr_total = float(T.astype(np.float64)[labels[:, :-1], labels[:, 1:]].sum())
    reg = 0.5 * float(np.sum(W.astype(np.float64) ** 2)) \
        + 0.5 * float(np.sum(T.astype(np.float64) ** 2))
    return in_maps, em_total + tr_total, reg, G64b


def host_finish(results, tr_total, reg):
    lz_raw = 0.0
    for c in range(NCORES):
        lz_raw += float(results[c]["LNS"].astype(np.float64).sum())
    logZ_total = lz_raw + B * (L - 1) * np.log(float(Y))
    loglik_sum = tr_total - logZ_total
    f = -C_REG * loglik_sum / B + reg
    return np.float32(f)


def kernel(X, labels, W, T, K):
    from concourse.bass_utils import run_bass_kernel_spmd

    nc = _build_program()
    in_maps, tr_total, reg, _ = host_prep(X, labels, W, T, K)
    last_err = None
    for _attempt in range(3):
        try:
            res = run_bass_kernel_spmd(nc, in_maps, list(range(NCORES)))
            out = host_finish(res.results, tr_total, reg)
            if np.isfinite(out):
                return out
            last_err = RuntimeError(f"non-finite result {out}")
        except Exception as e:   # transient device errors: retry
            last_err = e
    raise last_err


# revision 6
# speedup vs baseline: 1.0938x; 1.0938x over previous
"""Trainium2 Bass kernel for the CRF problem.

Math:
  feat = conv2d(X.view(-1,1,16,8), K, pad=2)  -> flatten      (B, L, D)
  e    = feat @ W                                              (B, L, Y)
Both are linear in X, so fold:  e = X @ G  with  G = C_K @ W  (D x Y),
C_K the 128x128 conv matrix built from the 5x5 kernel (host prep, tiny).

logZ via the *scaled* forward algorithm:
  A_0 = exp(e_0);  A_t = exp(e_t) * (A_{t-1} @ M),  M = exp(T)/Y
  logZ = log(sum_y A_{L-1}) + (L-1)*log(Y)
(per-step matvec with the constant 26x26 matrix M == one PE matmul).

Per-core layout (512 words/core = 4 groups x 128 words):
  partitions = 32*g + y (y<26, rows 26..31 zero-padded), free = words.
  e^T produced by matmul(lhsT=G64 fp8, rhs=X^T fp8 chunk) into psum
  (scaled by 64 for fp8 fidelity; the Exp activation rescales by 1/64).

The PE queue is in-order, so the serial chain rounds (matmul -> DVE
multiply round-trips of ~650ns) are emitted one at a time with ~2
e-matmuls slotted between them: the e-matmuls execute inside the window
where the next chain matmul waits on the DVE result, keeping the PE
busy and the tensor stream flowing at DMA pace.

em-score and the label-pair transition score are linear functionals of
(X, labels) with tiny outputs; the host computes them exactly while the
device handles the part that needs the recursion: logZ.  Device output
per core: LNS (4,128) = log(sum_y A) per (group, word).
"""

import numpy as np
import ml_dtypes

B, L, D, Y = 4096, 64, 128, 26
NCORES = 8
WPC = B // NCORES          # 512 words per core
NG, GW = 4, 128            # word groups per core
NTAU, TT = 8, 8            # t-blocks x t-per-block (NTAU*TT == L)
C_REG = 1000.0
GSCALE = 64.0              # fp8 pre-scale on G, undone in the Exp

_BF16 = ml_dtypes.bfloat16
_FP8 = ml_dtypes.float8_e4m3
_PROG = {}


def _conv_matrix(K5):
    """C[q, p]: flattened-input q contribution to flattened-output p."""
    H, Wd = 16, 8
    C = np.zeros((D, D), dtype=np.float64)
    for oh in range(H):
        for ow in range(Wd):
            p = oh * Wd + ow
            for kh in range(5):
                for kw in range(5):
                    ih, iw = oh + kh - 2, ow + kw - 2
                    if 0 <= ih < H and 0 <= iw < Wd:
                        C[ih * Wd + iw, p] = K5[kh, kw]
    return C


def _build_program(reps=1):
    if reps in _PROG:
        return _PROG[reps]
    import concourse.tile as tile
    import concourse.mybir as mybir
    from concourse import bacc
    from concourse.bass import ds, ts

    f32 = mybir.dt.float32
    bf16 = mybir.dt.bfloat16
    fp8 = mybir.dt.float8e4

    nc = bacc.Bacc("TRN2", target_bir_lowering=False, debug=False,
                   num_devices=NCORES)

    XT_d = nc.dram_tensor("XT", [D, WPC * L], fp8, kind="ExternalInput")
    G64_d = nc.dram_tensor("G64", [D, 32], fp8, kind="ExternalInput")
    BDM_d = nc.dram_tensor("BDM", [128, 128], bf16, kind="ExternalInput")
    BDMT_d = nc.dram_tensor("BDMT", [128, 128], bf16, kind="ExternalInput")
    ONES_d = nc.dram_tensor("ONES4", [128, 4], bf16, kind="ExternalInput")
    LNS_d = nc.dram_tensor("LNS", [4, GW], f32, kind="ExternalOutput")

    TMID = L // 2
    TAU_ORDER = [0, 7, 1, 6, 2, 5, 3, 4]
    # bank emission FIFO: need-order for the two chains
    BANK_FIFO = []
    for k in range(0, NTAU, 2):
        tf, tb_ = TAU_ORDER[k], TAU_ORDER[k + 1]
        BANK_FIFO += [(tf, 0), (tb_, 1), (tf, 1), (tb_, 0)]
    CHUNK = NG * GW * TT          # 4096 XT cols per tau
    with tile.TileContext(nc) as tc:
        with (
            tc.tile_pool(name="const", bufs=1) as cpool,
            tc.tile_pool(name="xt", bufs=3) as xtp,
            tc.tile_pool(name="e", bufs=NTAU) as ep,
            tc.tile_pool(name="a", bufs=6) as apool,
            tc.tile_pool(name="out", bufs=1) as opool,
            tc.tile_pool(name="pe", bufs=3, space="PSUM") as pep,
            tc.tile_pool(name="prf", bufs=1, space="PSUM") as prfp,
            tc.tile_pool(name="prb", bufs=1, space="PSUM") as prbp,
            tc.tile_pool(name="pl", bufs=1, space="PSUM") as plp,
        ):
            consts = {}

            def load_consts():
                g64 = cpool.tile([D, 32], fp8)
                nc.gpsimd.dma_start(g64[:], G64_d[:])
                bdm = cpool.tile([128, 128], bf16)
                nc.gpsimd.dma_start(bdm[:], BDM_d[:])
                bdmt = cpool.tile([128, 128], bf16)
                nc.gpsimd.dma_start(bdmt[:], BDMT_d[:])
                ones4 = cpool.tile([128, 4], bf16)
                nc.gpsimd.dma_start(ones4[:], ONES_d[:])
                consts.update(g64=g64, bdm=bdm, bdmt=bdmt, ones4=ones4)

            lns = opool.tile([4, GW], f32)

            for _rep in range(reps):
                e_tiles = {}
                staged = {}
                dma_done = set()

                def produce_dma(tau):
                    if tau in dma_done:
                        return
                    dma_done.add(tau)
                    xt = xtp.tile([D, CHUNK], fp8)
                    for half in range(2):
                        nc.sync.dma_start(
                            xt[:, ds(half * (CHUNK // 2), CHUNK // 2)],
                            XT_d[:, ds(tau * CHUNK + half * (CHUNK // 2),
                                       CHUNK // 2)])
                    staged[tau] = xt

                # per-bank emission state: psum tile + how many of the 4
                # group matmuls have been emitted; ACT emitted at 4
                bank_state = {}
                acts_done = set()

                def emit_mm(tau, tb):
                    """Emit one of the 4 group matmuls of bank (tau, tb)."""
                    key = (tau, tb)
                    if key not in bank_state:
                        if tau not in e_tiles:
                            e_tiles[tau] = ep.tile([128, GW * TT], bf16,
                                                   name="e_t", tag="e_t")
                        pe = pep.tile([128, 512], f32, name="pe_t", tag="pe_t")
                        bank_state[key] = [pe, 0]
                    pe, g = bank_state[key]
                    nc.tensor.matmul(
                        pe[32 * g:32 * g + 32, :],
                        consts["g64"][:],
                        staged[tau][:, ds((tb * NG + g) * 512, 512)],
                        start=True, stop=True,
                        tile_position=(0, 32 * g),
                    )
                    bank_state[key][1] += 1
                    if bank_state[key][1] == 4:
                        nc.scalar.activation(
                            e_tiles[tau][:, ds(tb * 512, 512)], pe[:],
                            mybir.ActivationFunctionType.Exp,
                            scale=1.0 / GSCALE,
                        )
                        acts_done.add(key)

                fifo_pos = [0]

                def emit_next_mms(n):
                    """Emit up to n pending e-matmuls in FIFO order."""
                    while n > 0 and fifo_pos[0] < 4 * len(BANK_FIFO):
                        bank = BANK_FIFO[fifo_pos[0] // 4]
                        # prefetch DMA two taus ahead of consumption
                        bi = fifo_pos[0] // 4
                        for future in BANK_FIFO[bi:bi + 5]:
                            produce_dma(future[0])
                        emit_mm(*bank)
                        fifo_pos[0] += 1
                        n -= 1

                def ensure_bank(tau, tb):
                    while (tau, tb) not in acts_done:
                        emit_next_mms(1)

                produce_dma(TAU_ORDER[0])
                if _rep == 0:
                    load_consts()
                produce_dma(TAU_ORDER[1])

                # banks for rounds 1..3 of both chains
                ensure_bank(0, 0)
                ensure_bank(NTAU - 1, 1)

                st_f = e_tiles[0][:, 0:GW]                       # alpha_0
                st_b = e_tiles[NTAU - 1][:, ds((TT - 1) * GW, GW)]  # g_63

                def eslice(t):
                    return e_tiles[t // TT][:, ds((t % TT) * GW, GW)]

                for r in range(1, TMID):
                    tfw, tbw = r, L - 1 - r
                    ensure_bank(tfw // TT, (tfw % TT) // 4)
                    ensure_bank(tbw // TT, (tbw % TT) // 4)
                    pr_f = prfp.tile([128, GW], f32, name="pr_f")
                    nc.tensor.matmul(pr_f[:], consts["bdm"][:], st_f,
                                     start=True, stop=True)
                    pr_b = prbp.tile([128, GW], f32, name="pr_b")
                    nc.tensor.matmul(pr_b[:], consts["bdmt"][:], st_b,
                                     start=True, stop=True)
                    emit_next_mms(2)
                    a_f = apool.tile([128, GW], bf16, name="a_f", tag="af")
                    nc.vector.tensor_mul(a_f[:], pr_f[:], eslice(tfw))
                    st_f = a_f[:]
                    a_b = apool.tile([128, GW], bf16, name="a_b", tag="ab")
                    nc.vector.tensor_mul(a_b[:], pr_b[:], eslice(tbw))
                    st_b = a_b[:]

                emit_next_mms(64)        # drain any stragglers (none normally)

                # beta_31 = M^T-apply(gamma_32), stays in psum
                pr_b = prbp.tile([128, GW], f32, name="pr_b")
                nc.tensor.matmul(pr_b[:], consts["bdmt"][:], st_b,
                                 start=True, stop=True)
                # logZ[w] = log( sum_y alpha_31 * beta_31 ) + 63*log(26)
                u = apool.tile([128, GW], bf16, tag="u")
                nc.vector.tensor_mul(u[:], pr_b[:], st_f)
                pl = plp.tile([4, GW], f32)
                nc.tensor.matmul(pl[:], consts["ones4"][:], u[:],
                                 start=True, stop=True)
                nc.scalar.activation(lns[:], pl[:],
                                     mybir.ActivationFunctionType.Ln)

            nc.sync.dma_start(LNS_d[:], lns[:])

    nc.compile()
    _PROG[reps] = nc
    return nc


def host_prep(X, labels, W, T, K):
    """Build per-core device inputs + host-side scalars."""
    X = np.asarray(X, dtype=np.float32)
    labels = np.asarray(labels).astype(np.int64)
    W = np.asarray(W, dtype=np.float32)
    T = np.asarray(T, dtype=np.float32)
    K5 = np.asarray(K, dtype=np.float64).reshape(5, 5)

    C = _conv_matrix(K5)
    G = C @ W.astype(np.float64)                    # (D, Y)
    G64b = np.zeros((D, 32), dtype=_FP8)
    G64b[:, :Y] = (G * GSCALE).astype(np.float32).astype(_FP8)

    M = (np.exp(T.astype(np.float64)) / Y).astype(np.float32)
    BDM = np.zeros((128, 128), dtype=_BF16)
    BDMT = np.zeros((128, 128), dtype=_BF16)
    for g in range(NG):
        BDM[32 * g:32 * g + Y, 32 * g:32 * g + Y] = M.astype(_BF16)
        BDMT[32 * g:32 * g + Y, 32 * g:32 * g + Y] = M.T.astype(_BF16)
    ONES = np.zeros((128, 4), dtype=_BF16)
    for g in range(NG):
        ONES[32 * g:32 * g + Y, g] = 1.0

    X8 = X.astype(_FP8)                             # (B, L, D)
    in_maps = []
    for c in range(NCORES):
        Xc = X8[c * WPC:(c + 1) * WPC]              # (512, 64, 128)
        # XT cols: tau-major | (tb,g) chunk | t'*128 + w'
        # Xc view (g, w', tau, tb, t', d) -> (d, tau, tb, g, t', w')
        Xv = Xc.reshape(NG, GW, NTAU, 2, TT // 2, D)
        XT = np.ascontiguousarray(
            Xv.transpose(5, 2, 3, 0, 4, 1)).reshape(D, WPC * L)
        in_maps.append({
            "XT": XT,
            "G64": G64b,
            "BDM": BDM,
            "BDMT": BDMT,
            "ONES4": ONES,
        })

    # host-side exact scalars: emission score (label-bucketed sum of X
    # contracted with G) + transition score
    lab = labels.ravel()
    X2 = X.reshape(-1, D)
    cnt = np.bincount(lab, minlength=Y)
    if cnt.min() > 0:
        perm = np.argsort(lab, kind="stable")
        starts = np.zeros(Y, dtype=np.int64)
        starts[1:] = np.cumsum(cnt)[:-1]
        S = np.add.reduceat(X2[perm], starts, axis=0)   # (Y, D)
    else:
        S = np.zeros((Y, D), dtype=np.float64)
        np.add.at(S, lab, X2)
    em_total = float((S.astype(np.float64) * G.T).sum())
    tr_total = float(T.astype(np.float64)[labels[:, :-1], labels[:, 1:]].sum())
    reg = 0.5 * float(np.sum(W.astype(np.float64) ** 2)) \
        + 0.5 * float(np.sum(T.astype(np.float64) ** 2))
    return in_maps, em_total + tr_total, reg, G64b


def host_finish(results, tr_total, reg):
    lz_raw = 0.0
    for c in range(NCORES):
        lz_raw += float(results[c]["LNS"].astype(np.float64).sum())
    logZ_total = lz_raw + B * (L - 1) * np.log(float(Y))
    loglik_sum = tr_total - logZ_total
    f = -C_REG * loglik_sum / B + reg
    return np.float32(f)


def kernel(X, labels, W, T, K):
    from concourse.bass_utils import run_bass_kernel_spmd

    nc = _build_program()
    in_maps, tr_total, reg, _ = host_prep(X, labels, W, T, K)
    last_err = None
    for _attempt in range(3):
        try:
            res = run_bass_kernel_spmd(nc, in_maps, list(range(NCORES)))
            out = host_finish(res.results, tr_total, reg)
            if np.isfinite(out):
                return out
            last_err = RuntimeError(f"non-finite result {out}")
        except Exception as e:   # transient device errors: retry
            last_err = e
    raise last_err
